# revision 32
# baseline (speedup 1.0000x reference)
"""Trainium2 Bass kernel for nn_CausalSelfAttention_78331613544603.

Tensor-parallel over heads across 8 NeuronCores (Megatron-style):
each core computes QKV for its 2 heads, runs causal attention for its
(batch, head) pairs, projects with its w_proj column-slice, and the
partial outputs are combined with chunked ReduceScatter collectives.
The host shards the weights and gathers the output shards.

Self-contained: only needs numpy + the concourse toolchain staged at
/opt/trn_rl_repo (also importable via the environment's PYTHONPATH).
"""

import math
import sys

import numpy as np

try:
    import concourse.bass as bass
except ImportError:
    sys.path.insert(0, "/opt/trn_rl_repo")
    import concourse.bass as bass

import concourse.mybir as mybir
import concourse.tile as tile
from concourse import bacc, bass_utils

F32 = mybir.dt.float32
F32R = mybir.dt.float32r
BF16 = mybir.dt.bfloat16
F16 = mybir.dt.float16

N_CORES = 8
HEADS = 16
HPC = HEADS // N_CORES  # heads per core = 2
HD = 256  # head dim
KV_CHANNELS = 128
NEG = -1.0e30

# q-super processing order: chunk j pairs supers (7-j, 3-j); chunk
# completions stay > the ~28us ReduceScatter apart and the final super
# (u=0) has the shortest softmax->PV->proj->RS drain chain
U_ORDER = [7, 3, 6, 2, 5, 1, 4, 0]
PAIRS = [(7, 3), (6, 2), (5, 1), (4, 0)]
CHUNK_OF_U = {u: j for j, p in enumerate(PAIRS) for u in p}
HALF_OF_U = {u: i for p in PAIRS for i, u in enumerate(p)}


class Cfg:
    def __init__(self, seq=2048, e=4096, out=2048):
        self.seq = seq  # sequence length
        self.batch = 2
        self.e = e  # input embedding dim (2*HIDDEN)
        self.out = out  # output dim (HIDDEN)
        self.ech = e // 128  # contraction chunks
        self.tok = seq * self.batch  # total tokens (batch-major)
        self.ntb = self.tok // 256  # qkv token blocks
        self.supers = seq // 256  # q super-tiles per (b,h)
        self.f_qk = HPC * HD * 2  # 1024 local q+k features
        self.f_v = HPC * HD  # 512 local v features
        self.nstg = self.tok // 512  # reduce-scatter chunks


def build_kernel(cfg: Cfg):
    nc = bacc.Bacc("TRN2", target_bir_lowering=False, debug=False,
                   num_devices=N_CORES)

    ECH = cfg.ech
    SEQ = cfg.seq
    TOK = cfg.tok
    OUT = cfg.out

    NTP = cfg.ntb // 2  # 512-token blocks

    # ---- kernel I/O ----
    hs5 = nc.dram_tensor("hs5", [NTP, ECH, 128, 512], F32R,
                         kind="ExternalInput")
    hsb = nc.dram_tensor("hsb", [NTP, ECH, 128, 512], F16,
                         kind="ExternalInput")
    wqk = nc.dram_tensor("wqk", [ECH, 128, cfg.f_qk], F32R,
                         kind="ExternalInput")
    wv = nc.dram_tensor("wv", [ECH, 128, cfg.f_v], F16,
                        kind="ExternalInput")
    wp = nc.dram_tensor("wp", [4, 128, OUT], F32R, kind="ExternalInput")
    maskm = nc.dram_tensor("maskm", [128, 1024], F32, kind="ExternalInput")
    ident = nc.dram_tensor("ident", [128, 128], F16, kind="ExternalInput")
    out_ext = nc.dram_tensor("out_ext", [cfg.nstg, 512 // N_CORES, OUT], F16,
                             kind="ExternalOutput")

    with tile.TileContext(nc) as tc:
        with (
            tc.tile_pool(name="dram", bufs=1, space="DRAM") as dramp,
        ):
            # bf16 hi/lo spill: hi = bf16(x), lo = bf16(x - hi). Both DVE
            # ops are bit-exact on fp32/PSUM reads, so the 3-term score
            # matmuls see clean 17-bit operands. (fp32r score matmuls are
            # 3x fewer instructions but the PE's float32r operand read is
            # lossy (~13.4 eff. bits) -> near-tie softmax flips push
            # rel_err over the 2e-2 gate. Measured, not theoretical.)
            qk_spill = dramp.tile([8, 2, 128, TOK], BF16, name="qk_spill")
            # 16-bit partials halve both the partial-store DMA and the
            # ReduceScatter time (21.5us vs 28.1us per chunk)
            partial_c = [dramp.tile([512, OUT], F16, name=f"partial{c}",
                                    tag=f"partial{c}")
                         for c in range(cfg.nstg)]
            # one RS per 512-token chunk: the collective is latency-bound
            # (~18us even at half size), so fewer, larger collectives win
            rs_out_c = [dramp.tile([512 // N_CORES, OUT], F16,
                                   name=f"rs_out{c}", tag=f"rs_out{c}")
                        for c in range(cfg.nstg)]

            with (
                tc.tile_pool(name="vres", bufs=1) as vres,
            ):
                # v for all tokens stays resident through attention
                v_all = vres.tile([128, TOK // 128, cfg.f_v], F16,
                                  name="v_all")

                kp_tiles = {}
                qs_tiles = {}

                def load_kp(b, h, pool, eng=None):
                    # chunked so the first QK block can start after the
                    # first 512 columns land
                    kp = pool.tile([128, 4, SEQ], BF16, name="kp",
                                   tag=f"kp{h}")
                    for ck in range(4):
                        for dc in range(2):
                            for hl in range(2):
                                (eng or nc.sync).dma_start(
                                    kp[:, 2 * dc + hl,
                                       ck * 512:(ck + 1) * 512],
                                    qk_spill[4 + 2 * h + dc, hl, :,
                                             b * SEQ + ck * 512:
                                             b * SEQ + (ck + 1) * 512])
                    kp_tiles[(b, h)] = kp

                qsp_cm = tc.tile_pool(name="qsp", bufs=3)
                qsp = qsp_cm.__enter__()

                def load_qs(b, h, u, eng=None):
                    qs = qsp.tile([128, 4, 256], BF16, name="qs", tag="qs")
                    for dc in range(2):
                        for hl in range(2):
                            (eng or nc.sync).dma_start(
                                qs[:, 2 * dc + hl, :],
                                qk_spill[2 * h + dc, hl, :,
                                         b * SEQ + u * 256:
                                         b * SEQ + (u + 1) * 256])
                    qs_tiles[(b, h, u)] = qs

                # wqk eh=0 j=0,1 tiles: prefetched during the V phase, live
                # through the QK phase (popped before attention opens)
                with tc.tile_pool(name="pre", bufs=1) as pre:
                    wqk_sb = {}

                    def load_w(j, eh, pool, halves=1, eng=None):
                        wqk_sb[(j, eh)] = pool.tile(
                            [128, ECH // 2, 256], F32R,
                            name=f"wqk_sb{j}_{eh}", tag=f"wqk_sb{j}_{eh}")
                        hq = ECH // 2 // halves
                        for v in range(halves):
                            (eng or nc.sync).dma_start(
                                wqk_sb[(j, eh)][:, v * hq:(v + 1) * hq, :],
                                wqk.ap()[eh * (ECH // 2) + v * hq:
                                         eh * (ECH // 2) + (v + 1) * hq, :,
                                         j * 256:(j + 1) * 256]
                                .rearrange("ec p f -> p ec f"),
                            )

                    # ================= phase 1: V projection ==============
                    with (
                        nc.named_scope("v_proj"),
                        tc.tile_pool(name="pvw", bufs=1) as pvw,
                        tc.tile_pool(name="p2hs", bufs=2) as p2hs,
                        tc.tile_pool(name="ps2", bufs=1, space="PSUM") as ps2,
                    ):
                        # wv in halves so the first matmuls start sooner
                        wv_sb = {}
                        for eh in range(2):
                            wv_sb[eh] = pvw.tile(
                                [128, ECH // 2, cfg.f_v], F16,
                                name=f"wv_sb{eh}", tag=f"wv_sb{eh}")
                            nsub = 4 if eh == 0 else 1
                            hq = ECH // 2 // nsub
                            for v in range(nsub):
                                nc.gpsimd.dma_start(
                                    wv_sb[eh][:, v * hq:(v + 1) * hq, :],
                                    wv.ap()[eh * (ECH // 2) + v * hq:
                                            eh * (ECH // 2) + (v + 1) * hq]
                                    .rearrange("ec p f -> p ec f"))
                        for tp in range(NTP):
                            ps_v = [
                                ps2.tile([128, cfg.f_v], F32, name="ps_v",
                                         tag=f"ps_v{tc_i}")
                                for tc_i in range(4)
                            ]
                            for eh in range(2):
                                hs_b = p2hs.tile([128, ECH // 2, 512], F16,
                                                 name="hs_b", tag="hs_b")
                                ns2 = 4 if (tp == 0 and eh == 0) else 1
                                h2 = ECH // 2 // ns2
                                for v2 in range(ns2):
                                    nc.sync.dma_start(
                                        hs_b[:, v2 * h2:(v2 + 1) * h2, :],
                                        hsb.ap()[tp,
                                                 eh * (ECH // 2) + v2 * h2:
                                                 eh * (ECH // 2)
                                                 + (v2 + 1) * h2]
                                        .rearrange("ec p t -> p ec t"),
                                    )
                                for tc_i in range(4):
                                    for el in range(ECH // 2):
                                        ec = eh * (ECH // 2) + el
                                        nc.tensor.matmul(
                                            ps_v[tc_i][:],
                                            hs_b[:, el,
                                                 tc_i * 128:(tc_i + 1) * 128],
                                            wv_sb[eh][:, el, :],
                                            start=(ec == 0),
                                            stop=(ec == ECH - 1),
                                        )
                            for tc_i in range(4):
                                g = tp * 4 + tc_i
                                nc.vector.tensor_copy(v_all[:, g, :],
                                                      ps_v[tc_i][:])
                            if tp == NTP - 3:
                                # prefetch the eh=0 QK weights on the idle
                                # DVE queue (SP is blocked by its own
                                # throttled hs stream)
                                for j in range(4):
                                    load_w(j, 0, pre, eng=nc.scalar)

                    # ================= phase 2: QK projection =============
                    with (
                        nc.named_scope("qk_proj"),
                        tc.tile_pool(name="p1", bufs=1) as p1,
                        tc.tile_pool(name="p1hs", bufs=2) as p1hs,
                        tc.tile_pool(name="p1st", bufs=2) as p1st,
                        tc.tile_pool(name="ps1", bufs=1, space="PSUM") as ps1,
                    ):
                        EPT = ECH // 4  # e-chunks per hs tile
                        for tp in range(NTP):
                            pst = [
                                ps1.tile([128, 512], F32, name=f"ps_qk{fc}",
                                         tag=f"ps_qk{fc}")
                                for fc in range(8)
                            ]
                            for sub in range(4):
                                eh = sub // 2
                                hs_t = p1hs.tile([128, EPT, 512], F32R,
                                                 name="hs_t", tag="hs_t")
                                heng = nc.scalar if (tp == 0 and sub < 2) \
                                    else nc.sync
                                heng.dma_start(
                                    hs_t[:],
                                    hs5.ap()[tp, sub * EPT:(sub + 1) * EPT]
                                    .rearrange("ec p t -> p ec t"),
                                )
                                if tp == 0 and sub < 2:
                                    # interleave the eh=1 weight loads with
                                    # the first hs tiles to avoid a burst
                                    load_w(2 * sub, 1, p1, halves=2,
                                           eng=nc.scalar)
                                    load_w(2 * sub + 1, 1, p1, halves=2,
                                           eng=nc.scalar)
                                for j in range(4):
                                    for el in range(EPT):
                                        ec = sub * EPT + el
                                        for half in range(2):
                                            fc = 2 * j + half
                                            nc.tensor.matmul(
                                                pst[fc][:],
                                                wqk_sb[(j, eh)][
                                                    :,
                                                    (sub % 2) * EPT + el,
                                                    half * 128:
                                                    (half + 1) * 128],
                                                hs_t[:, el, :],
                                                start=(ec == 0),
                                                stop=(ec == ECH - 1),
                                            )
                            for fc in range(8):
                                # bf16 hi/lo split; both ops are bit-exact
                                stg = p1st.tile([128, 2, 512], BF16,
                                                name="qkstg", tag="qkstg")
                                nc.vector.tensor_copy(stg[:, 0, :],
                                                      pst[fc][:])
                                nc.vector.tensor_tensor(
                                    stg[:, 1, :], pst[fc][:], stg[:, 0, :],
                                    mybir.AluOpType.subtract)
                                nc.sync.dma_start(
                                    qk_spill[fc, :, :,
                                             tp * 512:(tp + 1) * 512]
                                    .rearrange("hl p t -> p hl t"),
                                    stg[:])
                            if tp == 4:
                                # q-panels for the pipeline-fill stages of
                                # attention, on the idle Act queue
                                for (pb, ph, pu) in ((0, 0, U_ORDER[0]),
                                                     (0, 1, U_ORDER[0]),
                                                     (0, 0, U_ORDER[1])):
                                    load_qs(pb, ph, pu, eng=nc.scalar)

                # ====== phases 3+4: attention software-pipelined with ======
                # ====== per-batch output projection + ReduceScatter   ======
                with (
                    nc.named_scope("attn_proj"),
                    tc.tile_pool(name="attnc", bufs=1) as attnc,
                    tc.tile_pool(name="qkp", bufs=1) as qkp,
                    tc.tile_pool(name="strips", bufs=3) as strips,
                    tc.tile_pool(name="ptp", bufs=2) as ptp,
                    tc.tile_pool(name="statp", bufs=4) as statp,
                    tc.tile_pool(name="ytp", bufs=2) as ytp,
                    tc.tile_pool(name="pstrips", bufs=2) as pstrips,
                    tc.tile_pool(name="p4st", bufs=4) as p4st,
                    tc.tile_pool(name="ps3", bufs=2, space="PSUM") as ps3,
                    tc.tile_pool(name="ps3t", bufs=2, space="PSUM") as ps3t,
                    tc.tile_pool(name="ps3b", bufs=1, space="PSUM") as ps3b,
                    tc.tile_pool(name="ps3o", bufs=2, space="PSUM") as ps3o,
                ):
                    # constants go on the Pool queue in parallel with the
                    # q/k loads issued inside attention_body
                    mask_sb = attnc.tile([128, 1024], F32, name="mask_sb")
                    nc.gpsimd.dma_start(mask_sb[:], maskm.ap())
                    ident_sb = attnc.tile([128, 128], F16, name="ident_sb")
                    nc.gpsimd.dma_start(ident_sb[:], ident.ap())
                    wpt_all = attnc.tile([128, 4, OUT], F32R, name="wpt_all")
                    for fc in range(4):
                        nc.gpsimd.dma_start(wpt_all[:, fc, :], wp.ap()[fc])

                    def load_kp2(b, h, eng=None):
                        load_kp(b, h, qkp, eng=eng)

                    attention_body(nc, tc, cfg, v_all, qk_spill, partial_c,
                                   rs_out_c, out_ext, mask_sb, ident_sb,
                                   wpt_all,
                                   kp_tiles, load_kp2, load_qs, qs_tiles,
                                   strips, ptp,
                                   statp, ytp, pstrips, p4st, ps3, ps3t,
                                   ps3b, ps3o)
                qsp_cm.__exit__(None, None, None)

    nc.finalize()
    return nc


def attention_body(nc, tc, cfg, v_all, qk_spill, partial_c, rs_out_c,
                   out_ext, mask_sb, ident_sb, wpt_all, kp_tiles, load_kp,
                   load_qs, qs_tiles, strips, ptp, statp, ytp, pstrips,
                   p4st, ps3, ps3t, ps3b, ps3o):
    SEQ = cfg.seq
    OUT = cfg.out
    if True:
        if True:
            if True:
                n_ob = OUT // 512
                nstg_b = cfg.nstg // cfg.batch
                yt_t = {}
                def produce(b, h, u):
                    """QK for one q-super: S blocks -> masked strips."""
                    if (b, h) not in kp_tiles:
                        load_kp(b, h)
                    kp = kp_tiles[(b, h)]
                    if (b, h, u) not in qs_tiles:
                        load_qs(b, h, u)
                    qs = qs_tiles.pop((b, h, u))
                    nb = (u + 2) // 2
                    # PV only reads cols [0, (2u+2)*128); trim the last
                    # block to 256 wide when nb*512 overshoots by 256
                    rem = nb * 512 - (2 * u + 2) * 128
                    strip = [
                        strips.tile([128, SEQ], F32, name=f"strip{qt}",
                                    tag=f"strip{qt}")
                        for qt in range(2)
                    ]
                    for qt in range(2):
                        i = 2 * u + qt
                        d_jb = i // 4
                        for jb in range(nb):
                            w = 512 - rem if jb == nb - 1 else 512
                            dst = strip[qt][:, jb * 512:jb * 512 + w]
                            if jb > d_jb:
                                nc.scalar.copy(dst, mask_sb[:, 512:512 + w])
                                continue
                            # causal trim: the diagonal block only has
                            # (i%4+1)*128 valid cols; round up to 128s
                            wc = (i % 4 + 1) * 128 if jb == d_jb else w
                            ps_s = ps3.tile([128, 512], F32,
                                            name="ps_s", tag="ps_s")
                            TERMS = ((0, 0), (0, 1), (1, 0))
                            for idx in range(6):
                                ec, (qa, kb) = idx // 3, TERMS[idx % 3]
                                nc.tensor.matmul(
                                    ps_s[:, :wc],
                                    qs[:, 2 * ec + qa,
                                       qt * 128:(qt + 1) * 128],
                                    kp[:, 2 * ec + kb,
                                       jb * 512:jb * 512 + wc],
                                    start=(idx == 0),
                                    stop=(idx == 5),
                                )
                            if jb == d_jb:
                                off = 384 - 128 * (i % 4)
                                nc.vector.tensor_tensor(
                                    dst[:, :wc], ps_s[:, :wc],
                                    mask_sb[:, off:off + wc],
                                    mybir.AluOpType.add)
                                if wc < w:
                                    nc.scalar.copy(
                                        strip[qt][:, jb * 512 + wc:
                                                  jb * 512 + w],
                                        mask_sb[:, 512:512 + w - wc])
                            else:
                                nc.scalar.copy(dst, ps_s[:, :w])
                    return strip

                def consume(b, h, u, strip):
                    """softmax -> transpose P -> PV -> y^T drain."""
                    sl = CHUNK_OF_U[u]
                    half = HALF_OF_U[u]
                    if half == 0 and h == 0:
                        yt_t[(b, sl)] = ytp.tile([128, 4, 512], F32R,
                                                 name="yt", tag="yt")
                    nk = 2 * (u + 1)
                    Lp = nk * 128  # range PV reads (exp'd, masked -> 0)
                    pstrip = [
                        pstrips.tile([128, SEQ], F16, name=f"pstrip{qt}",
                                     tag=f"pstrip{qt}")
                        for qt in range(2)
                    ]
                    for qt in range(2):
                        Lv = (2 * u + qt + 1) * 128  # causally valid cols
                        negmax = statp.tile([128, 1], F32, name="negmax",
                                            tag="negmax")
                        nc.vector.reduce_max(
                            negmax[:], strip[qt][:, :Lv],
                            axis=mybir.AxisListType.X, negate=True)
                        zsum = statp.tile([128, 1], F32, name="zsum",
                                          tag="zsum")
                        nc.scalar.activation(
                            pstrip[qt][:, :Lp], strip[qt][:, :Lp],
                            mybir.ActivationFunctionType.Exp,
                            bias=negmax[:], scale=1.0, accum_out=zsum[:])
                        rz = statp.tile([128, 1], F32, name="rz", tag="rz")
                        nc.vector.reciprocal(rz[:], zsum[:])
                        nc.vector.tensor_scalar_mul(
                            pstrip[qt][:, :Lp], pstrip[qt][:, :Lp], rz[:])
                    ps_y = [
                        ps3b.tile([128, 256], F32, name=f"ps_y{dh}",
                                  tag=f"ps_y{dh}")
                        for dh in range(2)
                    ]
                    # two k-blocks per transpose batch: 4 transposes into
                    # one [128, 2, 256] PSUM tile, drained by a single copy
                    for cp in range(nk // 2):
                        ps_pt = ps3t.tile([128, 2, 256], F16,
                                          name="ps_pt", tag="ps_pt")
                        for ci in range(2):
                            c = 2 * cp + ci
                            for qt in range(2):
                                nc.tensor.transpose(
                                    ps_pt[:, ci, qt * 128:(qt + 1) * 128],
                                    pstrip[qt][:, c * 128:(c + 1) * 128],
                                    ident_sb[:])
                        pt_sb = ptp.tile([128, 2, 256], F16, name="pt_sb",
                                         tag="pt_sb")
                        nc.vector.tensor_copy(pt_sb[:], ps_pt[:])
                        for ci in range(2):
                            c = 2 * cp + ci
                            g = b * (SEQ // 128) + c
                            for dh in range(2):
                                nc.tensor.matmul(
                                    ps_y[dh][:],
                                    v_all[:, g, h * 256 + dh * 128:
                                          h * 256 + (dh + 1) * 128],
                                    pt_sb[:, ci, :],
                                    start=(c == 0),
                                    stop=(c == nk - 1),
                                )
                    for dh in range(2):
                        nc.scalar.copy(
                            yt_t[(b, sl)][:, 2 * h + dh,
                                          half * 256:(half + 1) * 256],
                            ps_y[dh][:])

                def proj_sts(b, sl, sts):
                    """project a 256-token half-chunk of y^T."""
                    stg = b * nstg_b + sl
                    for ob in range(n_ob):
                        for st in sts:
                            t0 = st * 128
                            ps_o = ps3o.tile([128, 512], F32, name="ps_o",
                                             tag="ps_o")
                            for fc in range(4):
                                nc.tensor.matmul(
                                    ps_o[:],
                                    yt_t[(b, sl)][:, fc, t0:t0 + 128],
                                    wpt_all[:, fc, ob * 512:(ob + 1) * 512],
                                    start=(fc == 0),
                                    stop=(fc == 3),
                                )
                            ost = p4st.tile([128, 512], F16, name="ost",
                                            tag="ost")
                            if (ob * 4 + st) % 2 == 0:
                                nc.vector.tensor_copy(ost[:], ps_o[:])
                            else:
                                nc.scalar.copy(ost[:], ps_o[:])
                            nc.sync.dma_start(
                                partial_c[stg][t0:t0 + 128,
                                               ob * 512:(ob + 1) * 512],
                                ost[:])

                def rs_chunk(b, sl):
                    stg = b * nstg_b + sl
                    nc.gpsimd.collective_compute(
                        "ReduceScatter",
                        mybir.AluOpType.add,
                        ins=[partial_c[stg].opt()],
                        outs=[rs_out_c[stg].opt()],
                        replica_groups=[list(range(N_CORES))],
                    )
                    # on the Pool queue: the wait for the RS hides behind
                    # the collective-core serialization of the next RS.
                    # The final chunk's copy is split across Pool and SP so
                    # its two halves run in parallel after the last RS.
                    if stg < cfg.nstg - 1:
                        nc.gpsimd.dma_start(out_ext.ap()[stg],
                                            rs_out_c[stg])
                    else:
                        half = 512 // N_CORES // 2
                        nc.gpsimd.dma_start(out_ext.ap()[stg, :half],
                                            rs_out_c[stg][:half])
                        nc.sync.dma_start(out_ext.ap()[stg, half:],
                                          rs_out_c[stg][half:])

                # Chunks pair a big and a small super (7-j with j) so the
                # per-chunk work is uniform: chunk completions land ~42us
                # apart (> the 28.1us ReduceScatter), so the RS chain never
                # queues, and the final drain chain is mid-size.
                stages = [(b, h, u)
                          for b in range(cfg.batch)
                          for u in U_ORDER
                          for h in range(HPC)]

                def after_consume(pb, ph, pu):
                    if ph == HPC - 1:
                        sl = CHUNK_OF_U[pu]
                        half = HALF_OF_U[pu]
                        proj_sts(pb, sl, (0, 1) if half == 0 else (2, 3))
                        if half == 1:
                            rs_chunk(pb, sl)

                # first stage's q-panel first, then the h=0 k-panel
                # chunks, then the rest of the pipeline-fill loads; all on
                # the idle Act queue so they dispatch ahead of the QK
                # phase's throttled SP stream
                load_kp(0, 0)
                for st in stages[:3]:
                    if st not in qs_tiles:
                        load_qs(*st)
                load_kp(0, 1)

                # depth-2 software pipeline: produce runs two stages ahead
                # of consume so the softmax chain (DVE/Act) of stage i
                # overlaps the PE work of stages i+1 / i+2.
                DEPTH = 2
                pending = []
                for stage in stages:
                    pending.append((stage, produce(*stage)))
                    if len(pending) > DEPTH:
                        (st, strip) = pending.pop(0)
                        consume(*st, strip)
                        after_consume(*st)
                for (st, strip) in pending:
                    consume(*st, strip)
                    after_consume(*st)


def prep_inputs(cfg: Cfg, hidden_states, w_qkv, w_proj):
    """Shard + lay out the full inputs for each of the 8 cores."""
    seq, batch, e = hidden_states.shape
    assert (seq, batch, e) == (cfg.seq, cfg.batch, cfg.e)
    import ml_dtypes
    hs_t = np.ascontiguousarray(
        hidden_states.transpose(1, 0, 2).reshape(cfg.tok, e).T
    )  # [e, tok], tokens batch-major
    ntp = cfg.ntb // 2
    # [ntp, ech, 128, 512]
    hs5 = np.ascontiguousarray(
        hs_t.reshape(cfg.ech, 128, ntp, 512).transpose(2, 0, 1, 3)
    ).astype(np.float32)
    hsb = hs5.astype(np.float16)

    scale = math.sqrt(math.sqrt(KV_CHANNELS))
    w3 = w_qkv.reshape(HEADS, 3, HD, e)
    mask = np.full((128, 1024), 0.0, dtype=np.float32)
    cols = np.arange(1024)[None, :]
    rows = np.arange(128)[:, None]
    mask[cols > 384 + rows] = NEG
    ident = np.eye(128, dtype=np.float16)

    in_maps = []
    for c in range(N_CORES):
        hsel = [2 * c, 2 * c + 1]
        w_q = (w3[hsel, 0] * scale).reshape(cfg.f_qk // 2, e)
        w_k = (w3[hsel, 1] * scale).reshape(cfg.f_qk // 2, e)
        w_v = w3[hsel, 2].reshape(cfg.f_v, e)
        wqk = np.concatenate([w_q, w_k], axis=0)  # [1024, e]
        wqk_t = np.ascontiguousarray(wqk.T.reshape(cfg.ech, 128, cfg.f_qk))
        wv_t = np.ascontiguousarray(w_v.T.reshape(cfg.ech, 128, cfg.f_v))
        wp_c = w_proj[:, c * cfg.f_v:(c + 1) * cfg.f_v]  # [OUT, 512]
        wp_t = np.ascontiguousarray(wp_c.T.reshape(4, 128, cfg.out))
        in_maps.append({
            "hs5": hs5,
            "hsb": hsb,
            "wqk": wqk_t.astype(np.float32),
            "wv": wv_t.astype(np.float16),
            "wp": wp_t.astype(np.float32),
            "maskm": mask,
            "ident": ident,
        })
    return in_maps


def assemble_output(cfg: Cfg, results):
    """Gather per-core ReduceScatter shards into the full [seq, b, out].

    Chunk (b, j) holds the tokens of supers PAIRS[j] = (ua, ub): rows
    0-255 are ua's 256 tokens, rows 256-511 are ub's.
    """
    rows = 512 // N_CORES
    full = np.empty((cfg.tok, cfg.out), dtype=np.float32)
    nstg_b = cfg.nstg // cfg.batch
    for b in range(cfg.batch):
        for j, (ua, ub) in enumerate(PAIRS):
            stg = b * nstg_b + j
            for r in range(N_CORES):
                shard = results[r]["out_ext"][stg]  # [rows, out]
                for k in range(rows):
                    row = r * rows + k
                    u = ua if row < 256 else ub
                    tok = b * cfg.seq + u * 256 + (row % 256)
                    full[tok] = shard[k]
    return np.ascontiguousarray(
        full.reshape(cfg.batch, cfg.seq, cfg.out).transpose(1, 0, 2))


_NC_CACHE = {}


def run(cfg: Cfg, hidden_states, w_qkv, w_proj, trace=False):
    key = (cfg.seq, cfg.e, cfg.out)
    if key not in _NC_CACHE:
        _NC_CACHE[key] = build_kernel(cfg)
    nc = _NC_CACHE[key]
    in_maps = prep_inputs(cfg, hidden_states, w_qkv, w_proj)
    res = bass_utils.run_bass_kernel_spmd(
        nc, in_maps, core_ids=list(range(N_CORES)), trace=trace)
    return assemble_output(cfg, res.results), res


def kernel(hidden_states, attention_mask, w_qkv, w_proj):
    cfg = Cfg()
    out, _ = run(cfg, np.asarray(hidden_states, dtype=np.float32),
                 np.asarray(w_qkv, dtype=np.float32),
                 np.asarray(w_proj, dtype=np.float32))
    return out

